# revision 14
# baseline (speedup 1.0000x reference)
"""BotSpot GNN message-passing kernel for 8 TRN2 NeuronCores (Bass/Tile).

Strategy (data-parallel over the 8192-edge minibatch, 1024 edges/core):
  - host pre-joins the 7 categorical embedding tables + continuous column
    into one bf16 feature row per device (113 features + ones column for
    bias folding, padded to 128); per-core tables are deduplicated to the
    ~100K device rows that core actually touches
  - device gathers neighbor feature rows edge-order via indirect DMA
    (128 rows x 256B per instruction - ONE gather per 128 neighbors
    instead of five in the naive layout)
  - PE transposes (batched 4 blocks per PSUM bank, vector psum->sbuf
    copies); xbar DMA-transpose is avoided because the tile scheduler
    serializes it against SWDGE gathers (~9us stall per supertile)
  - W_msg matmul with bias folded into the lhsT ones-row -> in-PSUM ReLU
    -> positional segmented mean over each edge's 100 neighbors
  - small per-edge MLP branches (channel, device, fusion, head) on-chip
"""

import numpy as np
import ml_dtypes

EMBED = 16
N_COMBIN, N_DEV, B, NB = 100000, 1000000, 8192, 100
DEV_CAPS = [50, 5, 30, 200, 500, 2000, 100]
D_DEV = 113
D_COMB = 46
D_DEV1, D_DEV2 = 67, 50
D_CH, D_MSG, D_FUS = 27, 67, 56
CAT_IN, D_C1, D_C2 = 106, 63, 31

N_CORES = 8
E_PER = B // N_CORES            # 1024 edges per core
TILE_E = 5                      # edges per 512-position tile
N_TILES = (E_PER + TILE_E - 1) // TILE_E  # 205
SLOTS = N_TILES * 4             # 820 slot-groups of 128 rows
SUP = 32                        # slots per supertile
PAD_E = N_TILES * TILE_E        # 1025 padded edge count
NSUPS = (SLOTS + SUP - 1) // SUP  # 26

BF16 = ml_dtypes.bfloat16


def _wrap_clamp_np(i, n):
    """jnp.ndarray[idx] semantics: negative wraps once, then clamp."""
    i = np.where(i < 0, i + n, i)
    return np.clip(i, 0, n - 1)


def _build_dev_features(device_feats, tabs):
    """[1M, 128] f32: [cont, lang, plat, os, country, carrier, brand,
    plat_os] + ones col at 113, zeros beyond."""
    n = device_feats.shape[0]
    out = np.zeros((n, 128), np.float32)
    out[:, 0] = device_feats[:, 0]
    cat = device_feats[:, 1:8].astype(np.int32)
    for c in range(7):
        cat[:, c] = _wrap_clamp_np(cat[:, c], DEV_CAPS[c])
    # reference order: lang, plat, os, country, carrier, brand, plat_os
    for j, c in enumerate([0, 1, 2, 3, 4, 5, 6]):
        out[:, 1 + 16 * j:17 + 16 * j] = tabs[c][cat[:, c]]
    out[:, 113] = 1.0
    return out


def _run(inputs, trace=False):
    import concourse.bass as bass
    import concourse.bacc as bacc
    import concourse.mybir as mybir
    import concourse.tile as tile
    from concourse.bass_utils import run_bass_kernel_spmd

    f32, bf16, i32 = mybir.dt.float32, mybir.dt.bfloat16, mybir.dt.int32

    combin_feats = np.asarray(inputs["combin_feats"], np.float32)
    device_feats = np.asarray(inputs["device_feats"], np.float32)
    channel_id_emb = np.asarray(inputs["channel_id_emb"], np.float32)
    tabs = [np.asarray(inputs[k], np.float32) for k in
            ("lang_emb", "plat_emb", "os_emb", "country_emb",
             "carrier_emb", "brand_emb", "plat_os_emb")]
    edges = np.asarray(inputs["edges"], np.int64)
    neibrs = np.asarray(inputs["sampled_neibrs"], np.int64)

    devX = _build_dev_features(device_feats, tabs)        # [1M, 128] f32

    def W(name):
        return np.asarray(inputs[name], np.float32)

    def lhsT_pad(w, kpad, bias=None):
        t = np.zeros((kpad, w.shape[0]), np.float32)
        t[: w.shape[1], :] = w.T
        if bias is not None:
            t[w.shape[1], :] = bias
        return t.astype(BF16)

    # bias folded into ones-row for the 113-wide inputs and the comb input
    Wmsg_l = lhsT_pad(W("W_msg"), 114, W("b_msg"))         # [114, 67]
    Wdev1_l = lhsT_pad(W("W_dev1"), 114, W("b_dev1"))      # [114, 67]
    Wch1_l = lhsT_pad(W("W_ch1"), 47, W("b_ch1"))          # [47, 27]
    Wdev2_l = lhsT_pad(W("W_dev2"), 67)                    # [67, 50]
    Wfus_ch_l = lhsT_pad(W("W_fus")[:, :D_CH], 27)         # [27, 56]
    Wfus_msg_l = lhsT_pad(W("W_fus")[:, D_CH:] / NB, 67)   # [67, 56] mean folded
    Wc1_f_l = lhsT_pad(W("W_c1")[:, :D_FUS], 56)           # [56, 63]
    Wc1_d_l = lhsT_pad(W("W_c1")[:, D_FUS:], 50)           # [50, 63]
    Wc2_l = lhsT_pad(W("W_c2"), 63)                        # [63, 31]
    Wc3_l = lhsT_pad(W("W_c3"), 31)                        # [31, 1]

    biases = np.zeros((128, 5), np.float32)
    for j, nm in enumerate(("b_dev2", "b_fus", "b_c1", "b_c2", "b_c3")):
        b = W(nm)
        biases[: len(b), j] = b

    # ---- host index prep (per core) ----
    e_comb = _wrap_clamp_np(edges[:, 0], N_COMBIN).astype(np.int64)
    e_dev = _wrap_clamp_np(edges[:, 1], N_DEV).astype(np.int64)
    nb_idx = _wrap_clamp_np(neibrs, N_DEV).astype(np.int64)  # [B, 100]

    loc_tabs, nbr_idx_np, edx_np, ecx_np = [], [], [], []
    for c in range(N_CORES):
        nb_c = nb_idx[c * E_PER:(c + 1) * E_PER]            # [1024, 100]
        uniq, inv = np.unique(nb_c.reshape(-1), return_inverse=True)
        loc_tabs.append(devX[uniq].astype(BF16))            # [U_c, 128]
        ce = np.zeros((PAD_E, NB), np.int32)
        ce[:E_PER] = inv.reshape(E_PER, NB)
        flat = np.zeros((N_TILES, 512), np.int32)
        flat[:, :500] = ce.reshape(N_TILES, 500)
        nbr_idx_np.append(flat.reshape(SLOTS, 128).T.copy())  # [128, SLOTS]

        edx_np.append(devX[e_dev[c * E_PER:(c + 1) * E_PER]].astype(BF16))
        ec = np.zeros((E_PER, 128), np.float32)
        crows = combin_feats[e_comb[c * E_PER:(c + 1) * E_PER]]  # [1024, 31]
        ec[:, :30] = crows[:, :30]
        cid = _wrap_clamp_np(crows[:, 30].astype(np.int32), N_COMBIN)
        ec[:, 30:46] = channel_id_emb[cid]
        ec[:, 46] = 1.0
        ecx_np.append(ec.astype(BF16))

    U_max = max(t.shape[0] for t in loc_tabs)
    for c in range(N_CORES):
        u = loc_tabs[c].shape[0]
        if u < U_max:
            loc_tabs[c] = np.concatenate(
                [loc_tabs[c], np.zeros((U_max - u, 128), BF16)])

    # ---- build bass kernel ----
    nc = bacc.Bacc("TRN2", target_bir_lowering=False, debug=False,
                   num_devices=N_CORES)

    dev_t = nc.dram_tensor("dev_t", [U_max, 128], bf16, kind="ExternalInput").ap()
    edx_t = nc.dram_tensor("edx_t", [E_PER, 128], bf16, kind="ExternalInput").ap()
    ecx_t = nc.dram_tensor("ecx_t", [E_PER, 128], bf16, kind="ExternalInput").ap()
    nbr_t = nc.dram_tensor("nbr_t", [128, SLOTS], i32, kind="ExternalInput").ap()
    wm_t = nc.dram_tensor("wm_t", [114, 67], bf16, kind="ExternalInput").ap()
    wd1_t = nc.dram_tensor("wd1_t", [114, 67], bf16, kind="ExternalInput").ap()
    wch_t = nc.dram_tensor("wch_t", [47, 27], bf16, kind="ExternalInput").ap()
    wd2_t = nc.dram_tensor("wd2_t", [67, 50], bf16, kind="ExternalInput").ap()
    wfc_t = nc.dram_tensor("wfc_t", [27, 56], bf16, kind="ExternalInput").ap()
    wfm_t = nc.dram_tensor("wfm_t", [67, 56], bf16, kind="ExternalInput").ap()
    wc1f_t = nc.dram_tensor("wc1f_t", [56, 63], bf16, kind="ExternalInput").ap()
    wc1d_t = nc.dram_tensor("wc1d_t", [50, 63], bf16, kind="ExternalInput").ap()
    wc2_t = nc.dram_tensor("wc2_t", [63, 31], bf16, kind="ExternalInput").ap()
    wc3_t = nc.dram_tensor("wc3_t", [31, 1], bf16, kind="ExternalInput").ap()
    bias_t = nc.dram_tensor("bias_t", [128, 5], f32, kind="ExternalInput").ap()
    out_t = nc.dram_tensor("out", [1, E_PER], f32, kind="ExternalOutput").ap()

    IOA = bass.IndirectOffsetOnAxis
    AX = mybir.AxisListType
    ALU = mybir.AluOpType
    ACTF = mybir.ActivationFunctionType

    from concourse.masks import make_identity

    with tile.TileContext(nc, trace_sim=False) as tc:
        with tc.tile_pool(name="const", bufs=1) as cpool, \
             tc.tile_pool(name="sbuf", bufs=3) as pool, \
             tc.tile_pool(name="big", bufs=1) as bigpool, \
             tc.tile_pool(name="psum", bufs=3, space="PSUM") as pp1, \
             tc.tile_pool(name="psumt", bufs=2, space="PSUM") as pptp, \
             tc.tile_pool(name="psume", bufs=2, space="PSUM") as ppe:

            def const(name, tt, shape, dtype):
                t = cpool.tile(shape, dtype, tag=name)
                nc.sync.dma_start(out=t[:], in_=tt[:])
                return t

            wm = const("wm", wm_t, [114, 67], bf16)
            wd1 = const("wd1", wd1_t, [114, 67], bf16)
            wch = const("wch", wch_t, [47, 27], bf16)
            wd2 = const("wd2", wd2_t, [67, 50], bf16)
            wfc = const("wfc", wfc_t, [27, 56], bf16)
            wfm = const("wfm", wfm_t, [67, 56], bf16)
            wc1f = const("wc1f", wc1f_t, [56, 63], bf16)
            wc1d = const("wc1d", wc1d_t, [50, 63], bf16)
            wc2 = const("wc2", wc2_t, [63, 31], bf16)
            wc3 = const("wc3", wc3_t, [31, 1], bf16)
            bias = const("bias", bias_t, [128, 5], f32)
            nbr_i = const("nbr", nbr_t, [128, SLOTS], i32)
            ident = cpool.tile([128, 128], bf16, tag="ident")
            make_identity(nc, ident[:])

            msg = bigpool.tile([67, PAD_E], f32)

            def transpose4(xview, t0, nblk, tag):
                """PE-transpose blocks t0..t0+nblk of x [128, s, 128] into a
                bf16 tile [128, nblk*128] via one PSUM bank + one copy."""
                tp = pptp.tile([128, 512], bf16, tag="tp", space="PSUM")
                with nc.allow_low_precision(reason="PE transpose, no accum"):
                    for c in range(nblk):
                        nc.tensor.transpose(out=tp[:, c * 128:(c + 1) * 128],
                                            in_=xview[:, t0 + c, :],
                                            identity=ident[:])
                xt = pool.tile([128, 512], bf16, tag=tag)
                nc.vector.tensor_copy(out=xt[:, :nblk * 128],
                                      in_=tp[:, :nblk * 128])
                return xt

            # ================= neighbor pipeline =================
            for sidx in range(NSUPS):
                s0 = sidx * SUP
                ns = min(SUP, SLOTS - s0)
                x = pool.tile([128, SUP * 128], bf16, tag="x")
                xv = x[:].rearrange("p (s f) -> p s f", f=128)
                for k in range(ns):
                    nc.gpsimd.indirect_dma_start(
                        out=xv[:, k, :], out_offset=None, in_=dev_t[:],
                        in_offset=IOA(ap=nbr_i[:, s0 + k:s0 + k + 1], axis=0))
                for t in range(ns // 4):
                    xt = transpose4(xv, 4 * t, 4, "xt")
                    p1 = pp1.tile([67, 512], f32, tag="p1", space="PSUM")
                    nc.tensor.matmul(out=p1[:], lhsT=wm[:114, :],
                                     rhs=xt[:114, :], start=True, stop=True)
                    nc.scalar.activation(out=p1[:], in_=p1[:], func=ACTF.Relu,
                                         bias=0.0, scale=1.0)
                    gt = sidx * 8 + t
                    nc.vector.tensor_reduce(
                        out=msg[:, gt * 5:(gt + 1) * 5],
                        in_=p1[:, :500].rearrange("p (e k) -> p e k", k=100),
                        axis=AX.X, op=ALU.add)

            # ================= edge branch =================
            xd = bigpool.tile([128, 8 * 128], bf16)
            xdv = xd[:].rearrange("p (s f) -> p s f", f=128)
            nc.sync.dma_start(
                out=xdv, in_=edx_t[:].rearrange("(s p) f -> p s f", p=128))
            xc = bigpool.tile([128, 8 * 128], bf16)
            xcv = xc[:].rearrange("p (s f) -> p s f", f=128)
            nc.sync.dma_start(
                out=xcv, in_=ecx_t[:].rearrange("(s p) f -> p s f", p=128))

            d1 = bigpool.tile([67, E_PER], bf16)
            d2 = bigpool.tile([50, E_PER], bf16)
            ch = bigpool.tile([27, E_PER], bf16)
            msgb = bigpool.tile([67, E_PER], bf16)
            nc.vector.tensor_copy(out=msgb[:], in_=msg[:, :E_PER])
            fus = bigpool.tile([56, E_PER], bf16)
            h1 = bigpool.tile([63, E_PER], bf16)
            h2 = bigpool.tile([31, E_PER], bf16)
            hout = bigpool.tile([1, E_PER], f32)
            for h in range(2):
                sl = slice(h * 512, h * 512 + 512)
                xdt = transpose4(xdv, 4 * h, 4, "xt")
                xct = transpose4(xcv, 4 * h, 4, "xt")
                p1 = ppe.tile([67, 512], f32, tag="ep", space="PSUM")
                nc.tensor.matmul(out=p1[:], lhsT=wd1[:114, :],
                                 rhs=xdt[:114, :], start=True, stop=True)
                nc.scalar.activation(out=d1[:, sl], in_=p1[:], func=ACTF.Relu,
                                     bias=0.0, scale=1.0)
                p2 = ppe.tile([50, 512], f32, tag="ep", space="PSUM")
                nc.tensor.matmul(out=p2[:], lhsT=wd2[:], rhs=d1[:67, sl],
                                 start=True, stop=True)
                nc.scalar.activation(out=d2[:, sl], in_=p2[:], func=ACTF.Relu,
                                     bias=bias[:50, 0:1], scale=1.0)
                p3 = ppe.tile([27, 512], f32, tag="ep", space="PSUM")
                nc.tensor.matmul(out=p3[:], lhsT=wch[:47, :],
                                 rhs=xct[:47, :], start=True, stop=True)
                nc.scalar.activation(out=ch[:, sl], in_=p3[:], func=ACTF.Relu,
                                     bias=0.0, scale=1.0)
                p4 = ppe.tile([56, 512], f32, tag="ep", space="PSUM")
                nc.tensor.matmul(out=p4[:], lhsT=wfc[:], rhs=ch[:27, sl],
                                 start=True, stop=False)
                nc.tensor.matmul(out=p4[:], lhsT=wfm[:], rhs=msgb[:67, sl],
                                 start=False, stop=True)
                nc.scalar.activation(out=fus[:, sl], in_=p4[:], func=ACTF.Relu,
                                     bias=bias[:56, 1:2], scale=1.0)
                p5 = ppe.tile([63, 512], f32, tag="ep", space="PSUM")
                nc.tensor.matmul(out=p5[:], lhsT=wc1f[:], rhs=fus[:56, sl],
                                 start=True, stop=False)
                nc.tensor.matmul(out=p5[:], lhsT=wc1d[:], rhs=d2[:50, sl],
                                 start=False, stop=True)
                nc.scalar.activation(out=h1[:, sl], in_=p5[:], func=ACTF.Relu,
                                     bias=bias[:63, 2:3], scale=1.0)
                p6 = ppe.tile([31, 512], f32, tag="ep", space="PSUM")
                nc.tensor.matmul(out=p6[:], lhsT=wc2[:], rhs=h1[:63, sl],
                                 start=True, stop=True)
                nc.scalar.activation(out=h2[:, sl], in_=p6[:], func=ACTF.Relu,
                                     bias=bias[:31, 3:4], scale=1.0)
                p7 = ppe.tile([1, 512], f32, tag="ep", space="PSUM")
                nc.tensor.matmul(out=p7[:], lhsT=wc3[:], rhs=h2[:31, sl],
                                 start=True, stop=True)
                nc.scalar.activation(out=hout[:, sl], in_=p7[:],
                                     func=ACTF.Identity, bias=bias[:1, 4:5],
                                     scale=1.0)
            nc.sync.dma_start(out=out_t[:], in_=hout[:])

    nc.compile()

    base = {
        "wm_t": np.asarray(Wmsg_l), "wd1_t": np.asarray(Wdev1_l),
        "wch_t": np.asarray(Wch1_l), "wd2_t": np.asarray(Wdev2_l),
        "wfc_t": np.asarray(Wfus_ch_l), "wfm_t": np.asarray(Wfus_msg_l),
        "wc1f_t": np.asarray(Wc1_f_l), "wc1d_t": np.asarray(Wc1_d_l),
        "wc2_t": np.asarray(Wc2_l), "wc3_t": np.asarray(Wc3_l),
        "bias_t": biases,
    }
    in_maps = []
    for c in range(N_CORES):
        m = dict(base)
        m["dev_t"] = loc_tabs[c]
        m["edx_t"] = edx_np[c]
        m["ecx_t"] = ecx_np[c]
        m["nbr_t"] = nbr_idx_np[c]
        in_maps.append(m)

    res = run_bass_kernel_spmd(nc, in_maps, core_ids=list(range(N_CORES)),
                               trace=trace)
    outs = [res.results[c]["out"].reshape(E_PER) for c in range(N_CORES)]
    full = np.concatenate(outs).reshape(B, 1).astype(np.float32)
    return full, res


def kernel(**inputs):
    out, _ = _run(inputs, trace=False)
    return out


# revision 15
# speedup vs baseline: 1.0005x; 1.0005x over previous
"""BotSpot GNN message-passing kernel for 8 TRN2 NeuronCores (Bass/Tile).

Strategy (data-parallel over the 8192-edge minibatch, 1024 edges/core):
  - host pre-joins the 7 categorical embedding tables + continuous column
    into one bf16 feature row per device (113 features + ones column for
    bias folding, padded to 128); per-core tables are deduplicated to the
    ~100K device rows that core actually touches
  - device gathers neighbor feature rows edge-order via indirect DMA
    (128 rows x 256B per instruction - ONE gather per 128 neighbors
    instead of five in the naive layout)
  - PE transposes (batched 4 blocks per PSUM bank, vector psum->sbuf
    copies); xbar DMA-transpose is avoided because the tile scheduler
    serializes it against SWDGE gathers (~9us stall per supertile)
  - W_msg matmul with bias folded into the lhsT ones-row -> in-PSUM ReLU
    -> positional segmented mean over each edge's 100 neighbors
  - small per-edge MLP branches (channel, device, fusion, head) on-chip
"""

import numpy as np
import ml_dtypes

EMBED = 16
N_COMBIN, N_DEV, B, NB = 100000, 1000000, 8192, 100
DEV_CAPS = [50, 5, 30, 200, 500, 2000, 100]
D_DEV = 113
D_COMB = 46
D_DEV1, D_DEV2 = 67, 50
D_CH, D_MSG, D_FUS = 27, 67, 56
CAT_IN, D_C1, D_C2 = 106, 63, 31

N_CORES = 8
E_PER = B // N_CORES            # 1024 edges per core
TILE_E = 5                      # edges per 512-position tile
N_TILES = (E_PER + TILE_E - 1) // TILE_E  # 205
SLOTS = N_TILES * 4             # 820 slot-groups of 128 rows
SUP = 32                        # slots per supertile
PAD_E = N_TILES * TILE_E        # 1025 padded edge count
NSUPS = (SLOTS + SUP - 1) // SUP  # 26

BF16 = ml_dtypes.bfloat16


def _wrap_clamp_np(i, n):
    """jnp.ndarray[idx] semantics: negative wraps once, then clamp."""
    i = np.where(i < 0, i + n, i)
    return np.clip(i, 0, n - 1)


def _build_dev_features(device_feats, tabs):
    """[1M, 128] f32: [cont, lang, plat, os, country, carrier, brand,
    plat_os] + ones col at 113, zeros beyond."""
    n = device_feats.shape[0]
    out = np.zeros((n, 128), np.float32)
    out[:, 0] = device_feats[:, 0]
    cat = device_feats[:, 1:8].astype(np.int32)
    for c in range(7):
        cat[:, c] = _wrap_clamp_np(cat[:, c], DEV_CAPS[c])
    # reference order: lang, plat, os, country, carrier, brand, plat_os
    for j, c in enumerate([0, 1, 2, 3, 4, 5, 6]):
        out[:, 1 + 16 * j:17 + 16 * j] = tabs[c][cat[:, c]]
    out[:, 113] = 1.0
    return out


def _run(inputs, trace=False):
    import concourse.bass as bass
    import concourse.bacc as bacc
    import concourse.mybir as mybir
    import concourse.tile as tile
    from concourse.bass_utils import run_bass_kernel_spmd

    f32, bf16, i32 = mybir.dt.float32, mybir.dt.bfloat16, mybir.dt.int32

    combin_feats = np.asarray(inputs["combin_feats"], np.float32)
    device_feats = np.asarray(inputs["device_feats"], np.float32)
    channel_id_emb = np.asarray(inputs["channel_id_emb"], np.float32)
    tabs = [np.asarray(inputs[k], np.float32) for k in
            ("lang_emb", "plat_emb", "os_emb", "country_emb",
             "carrier_emb", "brand_emb", "plat_os_emb")]
    edges = np.asarray(inputs["edges"], np.int64)
    neibrs = np.asarray(inputs["sampled_neibrs"], np.int64)

    devX = _build_dev_features(device_feats, tabs)        # [1M, 128] f32

    def W(name):
        return np.asarray(inputs[name], np.float32)

    def lhsT_pad(w, kpad, bias=None):
        t = np.zeros((kpad, w.shape[0]), np.float32)
        t[: w.shape[1], :] = w.T
        if bias is not None:
            t[w.shape[1], :] = bias
        return t.astype(BF16)

    # bias folded into ones-row for the 113-wide inputs and the comb input
    Wmsg_l = lhsT_pad(W("W_msg"), 114, W("b_msg"))         # [114, 67]
    Wdev1_l = lhsT_pad(W("W_dev1"), 114, W("b_dev1"))      # [114, 67]
    Wch1_l = lhsT_pad(W("W_ch1"), 47, W("b_ch1"))          # [47, 27]
    Wdev2_l = lhsT_pad(W("W_dev2"), 67)                    # [67, 50]
    Wfus_ch_l = lhsT_pad(W("W_fus")[:, :D_CH], 27)         # [27, 56]
    Wfus_msg_l = lhsT_pad(W("W_fus")[:, D_CH:] / NB, 67)   # [67, 56] mean folded
    Wc1_f_l = lhsT_pad(W("W_c1")[:, :D_FUS], 56)           # [56, 63]
    Wc1_d_l = lhsT_pad(W("W_c1")[:, D_FUS:], 50)           # [50, 63]
    Wc2_l = lhsT_pad(W("W_c2"), 63)                        # [63, 31]
    Wc3_l = lhsT_pad(W("W_c3"), 31)                        # [31, 1]

    biases = np.zeros((128, 5), np.float32)
    for j, nm in enumerate(("b_dev2", "b_fus", "b_c1", "b_c2", "b_c3")):
        b = W(nm)
        biases[: len(b), j] = b

    # ---- host index prep (per core) ----
    e_comb = _wrap_clamp_np(edges[:, 0], N_COMBIN).astype(np.int64)
    e_dev = _wrap_clamp_np(edges[:, 1], N_DEV).astype(np.int64)
    nb_idx = _wrap_clamp_np(neibrs, N_DEV).astype(np.int64)  # [B, 100]

    loc_tabs, nbr_idx_np, edx_np, ecx_np = [], [], [], []
    for c in range(N_CORES):
        nb_c = nb_idx[c * E_PER:(c + 1) * E_PER]            # [1024, 100]
        uniq, inv = np.unique(nb_c.reshape(-1), return_inverse=True)
        loc_tabs.append(devX[uniq].astype(BF16))            # [U_c, 128]
        ce = np.zeros((PAD_E, NB), np.int32)
        ce[:E_PER] = inv.reshape(E_PER, NB)
        flat = np.zeros((N_TILES, 512), np.int32)
        flat[:, :500] = ce.reshape(N_TILES, 500)
        nbr_idx_np.append(flat.reshape(SLOTS, 128).T.copy())  # [128, SLOTS]

        edx_np.append(devX[e_dev[c * E_PER:(c + 1) * E_PER]].astype(BF16))
        ec = np.zeros((E_PER, 128), np.float32)
        crows = combin_feats[e_comb[c * E_PER:(c + 1) * E_PER]]  # [1024, 31]
        ec[:, :30] = crows[:, :30]
        cid = _wrap_clamp_np(crows[:, 30].astype(np.int32), N_COMBIN)
        ec[:, 30:46] = channel_id_emb[cid]
        ec[:, 46] = 1.0
        ecx_np.append(ec.astype(BF16))

    U_max = max(t.shape[0] for t in loc_tabs)
    for c in range(N_CORES):
        u = loc_tabs[c].shape[0]
        if u < U_max:
            loc_tabs[c] = np.concatenate(
                [loc_tabs[c], np.zeros((U_max - u, 128), BF16)])

    # ---- build bass kernel ----
    nc = bacc.Bacc("TRN2", target_bir_lowering=False, debug=False,
                   num_devices=N_CORES)

    dev_t = nc.dram_tensor("dev_t", [U_max, 128], bf16, kind="ExternalInput").ap()
    edx_t = nc.dram_tensor("edx_t", [E_PER, 128], bf16, kind="ExternalInput").ap()
    ecx_t = nc.dram_tensor("ecx_t", [E_PER, 128], bf16, kind="ExternalInput").ap()
    nbr_t = nc.dram_tensor("nbr_t", [128, SLOTS], i32, kind="ExternalInput").ap()
    wm_t = nc.dram_tensor("wm_t", [114, 67], bf16, kind="ExternalInput").ap()
    wd1_t = nc.dram_tensor("wd1_t", [114, 67], bf16, kind="ExternalInput").ap()
    wch_t = nc.dram_tensor("wch_t", [47, 27], bf16, kind="ExternalInput").ap()
    wd2_t = nc.dram_tensor("wd2_t", [67, 50], bf16, kind="ExternalInput").ap()
    wfc_t = nc.dram_tensor("wfc_t", [27, 56], bf16, kind="ExternalInput").ap()
    wfm_t = nc.dram_tensor("wfm_t", [67, 56], bf16, kind="ExternalInput").ap()
    wc1f_t = nc.dram_tensor("wc1f_t", [56, 63], bf16, kind="ExternalInput").ap()
    wc1d_t = nc.dram_tensor("wc1d_t", [50, 63], bf16, kind="ExternalInput").ap()
    wc2_t = nc.dram_tensor("wc2_t", [63, 31], bf16, kind="ExternalInput").ap()
    wc3_t = nc.dram_tensor("wc3_t", [31, 1], bf16, kind="ExternalInput").ap()
    bias_t = nc.dram_tensor("bias_t", [128, 5], f32, kind="ExternalInput").ap()
    out_t = nc.dram_tensor("out", [1, E_PER], f32, kind="ExternalOutput").ap()

    IOA = bass.IndirectOffsetOnAxis
    AX = mybir.AxisListType
    ALU = mybir.AluOpType
    ACTF = mybir.ActivationFunctionType

    from concourse.masks import make_identity

    with tile.TileContext(nc, trace_sim=False) as tc:
        with tc.tile_pool(name="const", bufs=1) as cpool, \
             tc.tile_pool(name="sbuf", bufs=3) as pool, \
             tc.tile_pool(name="big", bufs=1) as bigpool, \
             tc.tile_pool(name="psum", bufs=3, space="PSUM") as pp1, \
             tc.tile_pool(name="psumt", bufs=2, space="PSUM") as pptp, \
             tc.tile_pool(name="psume", bufs=2, space="PSUM") as ppe:

            def const(name, tt, shape, dtype):
                t = cpool.tile(shape, dtype, tag=name)
                nc.sync.dma_start(out=t[:], in_=tt[:])
                return t

            wm = const("wm", wm_t, [114, 67], bf16)
            wd1 = const("wd1", wd1_t, [114, 67], bf16)
            wch = const("wch", wch_t, [47, 27], bf16)
            wd2 = const("wd2", wd2_t, [67, 50], bf16)
            wfc = const("wfc", wfc_t, [27, 56], bf16)
            wfm = const("wfm", wfm_t, [67, 56], bf16)
            wc1f = const("wc1f", wc1f_t, [56, 63], bf16)
            wc1d = const("wc1d", wc1d_t, [50, 63], bf16)
            wc2 = const("wc2", wc2_t, [63, 31], bf16)
            wc3 = const("wc3", wc3_t, [31, 1], bf16)
            bias = const("bias", bias_t, [128, 5], f32)
            nbr_i = const("nbr", nbr_t, [128, SLOTS], i32)
            ident = cpool.tile([128, 128], bf16, tag="ident")
            make_identity(nc, ident[:])

            msg = bigpool.tile([67, PAD_E], f32)

            def transpose4(xview, t0, nblk, tag):
                """PE-transpose blocks t0..t0+nblk of x [128, s, 128] into a
                bf16 tile [128, nblk*128] via one PSUM bank + one copy."""
                tp = pptp.tile([128, 512], bf16, tag="tp", space="PSUM")
                with nc.allow_low_precision(reason="PE transpose, no accum"):
                    for c in range(nblk):
                        nc.tensor.transpose(out=tp[:, c * 128:(c + 1) * 128],
                                            in_=xview[:, t0 + c, :],
                                            identity=ident[:])
                xt = pool.tile([128, 512], bf16, tag=tag)
                nc.scalar.copy(out=xt[:, :nblk * 128], in_=tp[:, :nblk * 128])
                return xt

            # ================= neighbor pipeline =================
            for sidx in range(NSUPS):
                s0 = sidx * SUP
                ns = min(SUP, SLOTS - s0)
                x = pool.tile([128, SUP * 128], bf16, tag="x")
                xv = x[:].rearrange("p (s f) -> p s f", f=128)
                for k in range(ns):
                    nc.gpsimd.indirect_dma_start(
                        out=xv[:, k, :], out_offset=None, in_=dev_t[:],
                        in_offset=IOA(ap=nbr_i[:, s0 + k:s0 + k + 1], axis=0))
                for t in range(ns // 4):
                    xt = transpose4(xv, 4 * t, 4, "xt")
                    p1 = pp1.tile([67, 512], f32, tag="p1", space="PSUM")
                    nc.tensor.matmul(out=p1[:], lhsT=wm[:114, :],
                                     rhs=xt[:114, :], start=True, stop=True)
                    nc.scalar.activation(out=p1[:], in_=p1[:], func=ACTF.Relu,
                                         bias=0.0, scale=1.0)
                    gt = sidx * 8 + t
                    nc.vector.tensor_reduce(
                        out=msg[:, gt * 5:(gt + 1) * 5],
                        in_=p1[:, :500].rearrange("p (e k) -> p e k", k=100),
                        axis=AX.X, op=ALU.add)

            # ================= edge branch =================
            xd = bigpool.tile([128, 8 * 128], bf16)
            xdv = xd[:].rearrange("p (s f) -> p s f", f=128)
            nc.sync.dma_start(
                out=xdv, in_=edx_t[:].rearrange("(s p) f -> p s f", p=128))
            xc = bigpool.tile([128, 8 * 128], bf16)
            xcv = xc[:].rearrange("p (s f) -> p s f", f=128)
            nc.sync.dma_start(
                out=xcv, in_=ecx_t[:].rearrange("(s p) f -> p s f", p=128))

            d1 = bigpool.tile([67, E_PER], bf16)
            d2 = bigpool.tile([50, E_PER], bf16)
            ch = bigpool.tile([27, E_PER], bf16)
            msgb = bigpool.tile([67, E_PER], bf16)
            nc.vector.tensor_copy(out=msgb[:], in_=msg[:, :E_PER])
            fus = bigpool.tile([56, E_PER], bf16)
            h1 = bigpool.tile([63, E_PER], bf16)
            h2 = bigpool.tile([31, E_PER], bf16)
            hout = bigpool.tile([1, E_PER], f32)
            for h in range(2):
                sl = slice(h * 512, h * 512 + 512)
                xdt = transpose4(xdv, 4 * h, 4, "xt")
                xct = transpose4(xcv, 4 * h, 4, "xt")
                p1 = ppe.tile([67, 512], f32, tag="ep", space="PSUM")
                nc.tensor.matmul(out=p1[:], lhsT=wd1[:114, :],
                                 rhs=xdt[:114, :], start=True, stop=True)
                nc.scalar.activation(out=d1[:, sl], in_=p1[:], func=ACTF.Relu,
                                     bias=0.0, scale=1.0)
                p2 = ppe.tile([50, 512], f32, tag="ep", space="PSUM")
                nc.tensor.matmul(out=p2[:], lhsT=wd2[:], rhs=d1[:67, sl],
                                 start=True, stop=True)
                nc.scalar.activation(out=d2[:, sl], in_=p2[:], func=ACTF.Relu,
                                     bias=bias[:50, 0:1], scale=1.0)
                p3 = ppe.tile([27, 512], f32, tag="ep", space="PSUM")
                nc.tensor.matmul(out=p3[:], lhsT=wch[:47, :],
                                 rhs=xct[:47, :], start=True, stop=True)
                nc.scalar.activation(out=ch[:, sl], in_=p3[:], func=ACTF.Relu,
                                     bias=0.0, scale=1.0)
                p4 = ppe.tile([56, 512], f32, tag="ep", space="PSUM")
                nc.tensor.matmul(out=p4[:], lhsT=wfc[:], rhs=ch[:27, sl],
                                 start=True, stop=False)
                nc.tensor.matmul(out=p4[:], lhsT=wfm[:], rhs=msgb[:67, sl],
                                 start=False, stop=True)
                nc.scalar.activation(out=fus[:, sl], in_=p4[:], func=ACTF.Relu,
                                     bias=bias[:56, 1:2], scale=1.0)
                p5 = ppe.tile([63, 512], f32, tag="ep", space="PSUM")
                nc.tensor.matmul(out=p5[:], lhsT=wc1f[:], rhs=fus[:56, sl],
                                 start=True, stop=False)
                nc.tensor.matmul(out=p5[:], lhsT=wc1d[:], rhs=d2[:50, sl],
                                 start=False, stop=True)
                nc.scalar.activation(out=h1[:, sl], in_=p5[:], func=ACTF.Relu,
                                     bias=bias[:63, 2:3], scale=1.0)
                p6 = ppe.tile([31, 512], f32, tag="ep", space="PSUM")
                nc.tensor.matmul(out=p6[:], lhsT=wc2[:], rhs=h1[:63, sl],
                                 start=True, stop=True)
                nc.scalar.activation(out=h2[:, sl], in_=p6[:], func=ACTF.Relu,
                                     bias=bias[:31, 3:4], scale=1.0)
                p7 = ppe.tile([1, 512], f32, tag="ep", space="PSUM")
                nc.tensor.matmul(out=p7[:], lhsT=wc3[:], rhs=h2[:31, sl],
                                 start=True, stop=True)
                nc.scalar.activation(out=hout[:, sl], in_=p7[:],
                                     func=ACTF.Identity, bias=bias[:1, 4:5],
                                     scale=1.0)
            nc.sync.dma_start(out=out_t[:], in_=hout[:])

    nc.compile()

    base = {
        "wm_t": np.asarray(Wmsg_l), "wd1_t": np.asarray(Wdev1_l),
        "wch_t": np.asarray(Wch1_l), "wd2_t": np.asarray(Wdev2_l),
        "wfc_t": np.asarray(Wfus_ch_l), "wfm_t": np.asarray(Wfus_msg_l),
        "wc1f_t": np.asarray(Wc1_f_l), "wc1d_t": np.asarray(Wc1_d_l),
        "wc2_t": np.asarray(Wc2_l), "wc3_t": np.asarray(Wc3_l),
        "bias_t": biases,
    }
    in_maps = []
    for c in range(N_CORES):
        m = dict(base)
        m["dev_t"] = loc_tabs[c]
        m["edx_t"] = edx_np[c]
        m["ecx_t"] = ecx_np[c]
        m["nbr_t"] = nbr_idx_np[c]
        in_maps.append(m)

    res = run_bass_kernel_spmd(nc, in_maps, core_ids=list(range(N_CORES)),
                               trace=trace)
    outs = [res.results[c]["out"].reshape(E_PER) for c in range(N_CORES)]
    full = np.concatenate(outs).reshape(B, 1).astype(np.float32)
    return full, res


def kernel(**inputs):
    out, _ = _run(inputs, trace=False)
    return out
